# revision 6
# baseline (speedup 1.0000x reference)
"""RNN-T Joiner kernel for Trainium2 (Bass/Tile), 8-core data-parallel over batch.

out[b,t,u,v] = (enc[b,t] @ We)[v] + (pred[b,u] @ Wp)[v] + bias[v]

Per core (one batch element):
  - PE (fp32): enc_proj [256,1024] and pred_b [65,1024] projections.
  - PE (fp32r): broadcast pred_b rows across the 128 t-partitions via one-hot
    selection matmuls into PSUM. Even u rows live at partitions 0-32, odd u
    rows at partitions 64-95, so consecutive matmuls alternate PE row groups
    and LDWEIGHTS overlaps in-flight MATMULs (64-deep reorder window).
  - DVE: one tensor_tensor add per output element (the mandatory PSUM->SBUF
    trip) producing staged output tiles.
  - HWDGE DMA: 10 uniform 6.8 MB contiguous stores (13 u's per block).
"""

import sys

sys.path.insert(0, "/opt/trn_rl_repo")

import numpy as np

B, T, U1, D, V = 8, 256, 65, 640, 1024
KC = D // 128  # 5 contraction chunks
UBLK = 13      # u's per output DMA block: 5 blocks x 13 = 65
NBLK = U1 // UBLK
NE = (U1 + 1) // 2  # 33 even u rows (0,2,..,64)
NO = U1 // 2        # 32 odd u rows (1,3,..,63)

_COMPILED = None


def _build():
    import concourse.bacc as bacc
    import concourse.tile as tile
    import concourse.mybir as mybir

    f32 = mybir.dt.float32
    f32r = mybir.dt.float32r

    nc = bacc.Bacc("TRN2", target_bir_lowering=False, debug=False, num_devices=8)

    encT = nc.dram_tensor("encT", [D, T], f32, kind="ExternalInput")
    # predT columns: even u's (0,2,..,64) then odd u's (1,3,..,63)
    predT = nc.dram_tensor("predT", [D, U1], f32, kind="ExternalInput")
    W = nc.dram_tensor("W", [2 * D, V], f32, kind="ExternalInput")
    bias = nc.dram_tensor("bias", [1, V], f32, kind="ExternalInput")
    ones = nc.dram_tensor("ones", [1, 128], f32, kind="ExternalInput")
    # packed one-hot: rows 0-32 select even u (identity33 x ones128),
    # rows 64-95 select odd u (identity32 x ones128)
    sel = nc.dram_tensor("sel", [128, NE * 128], f32r, kind="ExternalInput")
    out = nc.dram_tensor("out", [T, U1 * V], f32, kind="ExternalOutput")

    with tile.TileContext(nc) as tc:
        with tc.tile_pool(name="consts", bufs=1) as cp:
            sel_sb = cp.tile([128, NE * 128], f32r, tag="sel")
            nc.sync.dma_start(sel_sb[:], sel[:])
            pred_sp = cp.tile([128, V], f32r, tag="pred_sp")
            enc_dup = []
            for tt in range(2):
                t_ = cp.tile([128, 2 * V], f32, tag=f"enc_dup{tt}")
                enc_dup.append(t_)

            with tc.tile_pool(name="wpool", bufs=1) as wp:
                predT_sb = []
                Wp_sb = []
                encT_sb = []
                We_sb = []
                for c in range(KC):
                    t_ = wp.tile([128, U1], f32, tag=f"predT{c}")
                    nc.sync.dma_start(t_[:], predT[c * 128:(c + 1) * 128, :])
                    predT_sb.append(t_)
                    t_ = wp.tile([128, V], f32, tag=f"Wp{c}")
                    nc.sync.dma_start(t_[:], W[D + c * 128:D + (c + 1) * 128, :])
                    Wp_sb.append(t_)
                bias_sb = wp.tile([1, V], f32, tag="bias")
                nc.sync.dma_start(bias_sb[:], bias[:])
                ones_sb = wp.tile([1, 128], f32, tag="ones")
                nc.sync.dma_start(ones_sb[:], ones[:])
                for c in range(KC):
                    t_ = wp.tile([128, T], f32, tag=f"encT{c}")
                    nc.sync.dma_start(t_[:], encT[c * 128:(c + 1) * 128, :])
                    encT_sb.append(t_)
                    t_ = wp.tile([128, V], f32, tag=f"We{c}")
                    nc.sync.dma_start(t_[:], W[c * 128:(c + 1) * 128, :])
                    We_sb.append(t_)

                # ---- setup: projections (fp32 PE matmuls) ----
                with tc.tile_pool(name="spsum", bufs=2, space="PSUM") as sp:
                    ps_p = sp.tile([128, V], f32, tag="ps")
                    for vt in range(2):
                        vs = slice(vt * 512, (vt + 1) * 512)
                        for c in range(KC):
                            nc.tensor.matmul(
                                ps_p[0:NE, vs], predT_sb[c][:, 0:NE],
                                Wp_sb[c][:, vs], start=(c == 0), stop=False)
                        nc.tensor.matmul(
                            ps_p[0:NE, vs], ones_sb[0:1, 0:NE], bias_sb[0:1, vs],
                            start=False, stop=True)
                    for vt in range(2):
                        vs = slice(vt * 512, (vt + 1) * 512)
                        for c in range(KC):
                            nc.tensor.matmul(
                                ps_p[64:64 + NO, vs], predT_sb[c][:, NE:U1],
                                Wp_sb[c][:, vs], start=(c == 0), stop=False)
                        nc.tensor.matmul(
                            ps_p[64:64 + NO, vs], ones_sb[0:1, 0:NO], bias_sb[0:1, vs],
                            start=False, stop=True)
                    nc.vector.tensor_copy(pred_sp[0:NE, :], ps_p[0:NE, :])
                    nc.vector.tensor_copy(pred_sp[64:64 + NO, :], ps_p[64:64 + NO, :])

                    for tt in range(2):
                        ts_ = slice(tt * 128, (tt + 1) * 128)
                        ps_e = sp.tile([128, V], f32, tag="pse")
                        for vt in range(2):
                            vs = slice(vt * 512, (vt + 1) * 512)
                            for c in range(KC):
                                nc.tensor.matmul(
                                    ps_e[:, vs], encT_sb[c][:, ts_], We_sb[c][:, vs],
                                    start=(c == 0), stop=(c == KC - 1))
                        nc.vector.tensor_copy(enc_dup[tt][:, 0:V], ps_e[:])
                        nc.vector.tensor_copy(enc_dup[tt][:, V:2 * V], ps_e[:])

            def bcast_mm(ps_ap, u, vt):
                # one [128,512] slice of pred_b[u] broadcast to all partitions
                vs = slice(vt * 512, (vt + 1) * 512)
                if u % 2 == 0:
                    nc.tensor.matmul(
                        ps_ap, sel_sb[0:NE, (u // 2) * 128:(u // 2 + 1) * 128],
                        pred_sp[0:NE, vs], start=True, stop=True)
                else:
                    nc.tensor.matmul(
                        ps_ap, sel_sb[64:64 + NO, (u // 2) * 128:(u // 2 + 1) * 128],
                        pred_sp[64:64 + NO, vs], start=True, stop=True)

            # ---- main loop: broadcast-add-store ----
            with tc.tile_pool(name="outp", bufs=2) as op_, \
                 tc.tile_pool(name="mpsum", bufs=2, space="PSUM") as mp:
                for tt in range(2):
                    rs = slice(tt * 128, (tt + 1) * 128)
                    for blk in range(NBLK):
                        u0 = blk * UBLK
                        stage = op_.tile([128, UBLK * V], f32, tag="stage")
                        for pair in range(UBLK // 2):
                            ua = u0 + 2 * pair
                            ps = mp.tile([128, 2048], f32, tag="mps")
                            # emission order alternates even/odd u -> row groups
                            bcast_mm(ps[:, 0:512], ua, 0)
                            bcast_mm(ps[:, 1024:1536], ua + 1, 0)
                            bcast_mm(ps[:, 512:1024], ua, 1)
                            bcast_mm(ps[:, 1536:2048], ua + 1, 1)
                            nc.vector.tensor_add(
                                stage[:, pair * 2048:(pair + 1) * 2048],
                                enc_dup[tt][:], ps[:])
                        # 13th u of the block
                        ul = u0 + UBLK - 1
                        ps = mp.tile([128, 2048], f32, tag="mps")
                        bcast_mm(ps[:, 0:512], ul, 0)
                        bcast_mm(ps[:, 512:1024], ul, 1)
                        nc.vector.tensor_add(
                            stage[:, (UBLK - 1) * V:UBLK * V],
                            enc_dup[tt][:, 0:V], ps[:, 0:V])
                        nc.sync.dma_start(
                            out[rs, u0 * V:(u0 + UBLK) * V], stage[:])

    nc.compile()
    return nc


def _get_compiled():
    global _COMPILED
    if _COMPILED is None:
        _COMPILED = _build()
    return _COMPILED


def _in_maps(encoder_out, predictor_out, W, b):
    sel = np.zeros((128, NE * 128), dtype=np.float32)
    for r in range(NE):
        sel[r, r * 128:(r + 1) * 128] = 1.0      # selects even u = 2r
    for r in range(NO):
        sel[64 + r, r * 128:(r + 1) * 128] = 1.0  # selects odd u = 2r+1
    ones = np.ones((1, 128), dtype=np.float32)
    bias = np.ascontiguousarray(b.reshape(1, V).astype(np.float32))
    Wc = np.ascontiguousarray(W.astype(np.float32))
    eo = list(range(0, U1, 2)) + list(range(1, U1, 2))
    maps = []
    for i in range(B):
        pT = predictor_out[i].T.astype(np.float32)  # [D, U1]
        maps.append({
            "encT": np.ascontiguousarray(encoder_out[i].T.astype(np.float32)),
            "predT": np.ascontiguousarray(pT[:, eo]),
            "W": Wc,
            "bias": bias,
            "ones": ones,
            "sel": sel,
        })
    return maps


def run(encoder_out, predictor_out, W, b, trace=False, tmpdir=None):
    from concourse.bass_utils import run_bass_kernel_spmd

    nc = _get_compiled()
    maps = _in_maps(encoder_out, predictor_out, W, b)
    res = run_bass_kernel_spmd(
        nc, maps, list(range(B)), trace=trace,
        **({"tmpdir": tmpdir} if tmpdir else {}))
    outs = np.stack([res.results[i]["out"].reshape(T, U1, V) for i in range(B)])
    return outs, res


def kernel(encoder_out, predictor_out, W, b):
    outs, _ = run(encoder_out, predictor_out, W, b)
    return outs


# revision 7
# speedup vs baseline: 1.0238x; 1.0238x over previous
"""RNN-T Joiner kernel for Trainium2 (Bass/Tile), 8-core data-parallel over batch.

out[b,t,u,v] = (enc[b,t] @ We)[v] + (pred[b,u] @ Wp)[v] + bias[v]

Per core (one batch element):
  - PE (fp32): enc_proj [256,1024] and pred_b [65,1024] projections.
  - PE (fp32r): broadcast pred_b rows across the 128 t-partitions via one-hot
    selection matmuls into PSUM. Even u rows live at partitions 0-32, odd u
    rows at partitions 64-95, so consecutive matmuls alternate PE row groups
    and LDWEIGHTS overlaps in-flight MATMULs (64-deep reorder window).
  - DVE: one tensor_tensor add per output element (the mandatory PSUM->SBUF
    trip) producing staged output tiles.
  - HWDGE DMA: 10 uniform 6.8 MB contiguous stores (13 u's per block).
"""

import sys

sys.path.insert(0, "/opt/trn_rl_repo")

import numpy as np

B, T, U1, D, V = 8, 256, 65, 640, 1024
KC = D // 128  # 5 contraction chunks
UBLK = 13      # u's per output DMA block: 5 blocks x 13 = 65
NBLK = U1 // UBLK
NE = (U1 + 1) // 2  # 33 even u rows (0,2,..,64)
NO = U1 // 2        # 32 odd u rows (1,3,..,63)

_COMPILED = None


def _build():
    import concourse.bacc as bacc
    import concourse.tile as tile
    import concourse.mybir as mybir

    f32 = mybir.dt.float32
    f32r = mybir.dt.float32r

    nc = bacc.Bacc("TRN2", target_bir_lowering=False, debug=False, num_devices=8)

    encT = nc.dram_tensor("encT", [D, T], f32, kind="ExternalInput")
    # predT columns: even u's (0,2,..,64) then odd u's (1,3,..,63)
    predT = nc.dram_tensor("predT", [D, U1], f32, kind="ExternalInput")
    W = nc.dram_tensor("W", [2 * D, V], f32, kind="ExternalInput")
    bias = nc.dram_tensor("bias", [1, V], f32, kind="ExternalInput")
    ones = nc.dram_tensor("ones", [1, 128], f32, kind="ExternalInput")
    # packed one-hot: rows 0-32 select even u (identity33 x ones128),
    # rows 64-95 select odd u (identity32 x ones128)
    sel = nc.dram_tensor("sel", [128, NE * 128], f32r, kind="ExternalInput")
    out = nc.dram_tensor("out", [T, U1 * V], f32, kind="ExternalOutput")

    with tile.TileContext(nc) as tc:
        with tc.tile_pool(name="consts", bufs=1) as cp:
            sel_sb = cp.tile([128, NE * 128], f32r, tag="sel")
            nc.sync.dma_start(sel_sb[:], sel[:])
            pred_sp = cp.tile([128, V], f32r, tag="pred_sp")
            enc_dup = []
            for tt in range(2):
                t_ = cp.tile([128, 2 * V], f32, tag=f"enc_dup{tt}")
                enc_dup.append(t_)

            with tc.tile_pool(name="wpool", bufs=1) as wp:
                predT_sb = []
                Wp_sb = []
                encT_sb = []
                We_sb = []
                for c in range(KC):
                    t_ = wp.tile([128, U1], f32, tag=f"predT{c}")
                    nc.sync.dma_start(t_[:], predT[c * 128:(c + 1) * 128, :])
                    predT_sb.append(t_)
                    t_ = wp.tile([128, V], f32, tag=f"Wp{c}")
                    nc.sync.dma_start(t_[:], W[D + c * 128:D + (c + 1) * 128, :])
                    Wp_sb.append(t_)
                bias_sb = wp.tile([1, V], f32, tag="bias")
                nc.sync.dma_start(bias_sb[:], bias[:])
                ones_sb = wp.tile([1, 128], f32, tag="ones")
                nc.sync.dma_start(ones_sb[:], ones[:])
                for c in range(KC):
                    t_ = wp.tile([128, T], f32, tag=f"encT{c}")
                    nc.sync.dma_start(t_[:], encT[c * 128:(c + 1) * 128, :])
                    encT_sb.append(t_)
                    t_ = wp.tile([128, V], f32, tag=f"We{c}")
                    nc.sync.dma_start(t_[:], W[c * 128:(c + 1) * 128, :])
                    We_sb.append(t_)

                # ---- setup: projections (fp32 PE matmuls) ----
                with tc.tile_pool(name="spsum", bufs=2, space="PSUM") as sp:
                    ps_p = sp.tile([128, V], f32, tag="ps")
                    for vt in range(2):
                        vs = slice(vt * 512, (vt + 1) * 512)
                        for c in range(KC):
                            nc.tensor.matmul(
                                ps_p[0:NE, vs], predT_sb[c][:, 0:NE],
                                Wp_sb[c][:, vs], start=(c == 0), stop=False)
                        nc.tensor.matmul(
                            ps_p[0:NE, vs], ones_sb[0:1, 0:NE], bias_sb[0:1, vs],
                            start=False, stop=True)
                    for vt in range(2):
                        vs = slice(vt * 512, (vt + 1) * 512)
                        for c in range(KC):
                            nc.tensor.matmul(
                                ps_p[64:64 + NO, vs], predT_sb[c][:, NE:U1],
                                Wp_sb[c][:, vs], start=(c == 0), stop=False)
                        nc.tensor.matmul(
                            ps_p[64:64 + NO, vs], ones_sb[0:1, 0:NO], bias_sb[0:1, vs],
                            start=False, stop=True)
                    nc.vector.tensor_copy(pred_sp[0:NE, :], ps_p[0:NE, :])
                    nc.vector.tensor_copy(pred_sp[64:64 + NO, :], ps_p[64:64 + NO, :])

                    for tt in range(2):
                        ts_ = slice(tt * 128, (tt + 1) * 128)
                        ps_e = sp.tile([128, V], f32, tag="pse")
                        for vt in range(2):
                            vs = slice(vt * 512, (vt + 1) * 512)
                            for c in range(KC):
                                nc.tensor.matmul(
                                    ps_e[:, vs], encT_sb[c][:, ts_], We_sb[c][:, vs],
                                    start=(c == 0), stop=(c == KC - 1))
                        nc.vector.tensor_copy(enc_dup[tt][:, 0:V], ps_e[:])
                        nc.vector.tensor_copy(enc_dup[tt][:, V:2 * V], ps_e[:])

            def bcast_mm(ps_ap, u, vt):
                # one [128,512] slice of pred_b[u] broadcast to all partitions
                vs = slice(vt * 512, (vt + 1) * 512)
                if u % 2 == 0:
                    nc.tensor.matmul(
                        ps_ap, sel_sb[0:NE, (u // 2) * 128:(u // 2 + 1) * 128],
                        pred_sp[0:NE, vs], start=True, stop=True)
                else:
                    nc.tensor.matmul(
                        ps_ap, sel_sb[64:64 + NO, (u // 2) * 128:(u // 2 + 1) * 128],
                        pred_sp[64:64 + NO, vs], start=True, stop=True)

            # ---- main loop: broadcast-add-store ----
            # psum broadcast tiles are identical for both t-halves: compute
            # once, add into both t-stages (halves PE work).
            with tc.tile_pool(name="outp", bufs=2) as op_, \
                 tc.tile_pool(name="mpsum", bufs=2, space="PSUM") as mp:
                for blk in range(8):
                    u0 = blk * 8
                    stage0 = op_.tile([128, 8 * V], f32, tag="stage0")
                    stage1 = op_.tile([128, 8 * V], f32, tag="stage1")
                    for pair in range(4):
                        ua = u0 + 2 * pair
                        ps = mp.tile([128, 2048], f32, tag="mps")
                        bcast_mm(ps[:, 0:512], ua, 0)
                        bcast_mm(ps[:, 1024:1536], ua + 1, 0)
                        bcast_mm(ps[:, 512:1024], ua, 1)
                        bcast_mm(ps[:, 1536:2048], ua + 1, 1)
                        nc.vector.tensor_add(
                            stage0[:, pair * 2048:(pair + 1) * 2048],
                            enc_dup[0][:], ps[:])
                        nc.vector.tensor_add(
                            stage1[:, pair * 2048:(pair + 1) * 2048],
                            enc_dup[1][:], ps[:])
                    nc.sync.dma_start(out[0:128, u0 * V:(u0 + 8) * V], stage0[:])
                    nc.sync.dma_start(out[128:256, u0 * V:(u0 + 8) * V], stage1[:])
                # tail u = 64
                u = U1 - 1
                stage0 = op_.tile([128, 8 * V], f32, tag="stage0")
                stage1 = op_.tile([128, 8 * V], f32, tag="stage1")
                ps = mp.tile([128, 2048], f32, tag="mps")
                bcast_mm(ps[:, 0:512], u, 0)
                bcast_mm(ps[:, 512:1024], u, 1)
                nc.vector.tensor_add(stage0[:, 0:V], enc_dup[0][:, 0:V], ps[:, 0:V])
                nc.vector.tensor_add(stage1[:, 0:V], enc_dup[1][:, 0:V], ps[:, 0:V])
                nc.sync.dma_start(out[0:128, u * V:(u + 1) * V], stage0[:, 0:V])
                nc.sync.dma_start(out[128:256, u * V:(u + 1) * V], stage1[:, 0:V])

    nc.compile()
    return nc


def _get_compiled():
    global _COMPILED
    if _COMPILED is None:
        _COMPILED = _build()
    return _COMPILED


def _in_maps(encoder_out, predictor_out, W, b):
    sel = np.zeros((128, NE * 128), dtype=np.float32)
    for r in range(NE):
        sel[r, r * 128:(r + 1) * 128] = 1.0      # selects even u = 2r
    for r in range(NO):
        sel[64 + r, r * 128:(r + 1) * 128] = 1.0  # selects odd u = 2r+1
    ones = np.ones((1, 128), dtype=np.float32)
    bias = np.ascontiguousarray(b.reshape(1, V).astype(np.float32))
    Wc = np.ascontiguousarray(W.astype(np.float32))
    eo = list(range(0, U1, 2)) + list(range(1, U1, 2))
    maps = []
    for i in range(B):
        pT = predictor_out[i].T.astype(np.float32)  # [D, U1]
        maps.append({
            "encT": np.ascontiguousarray(encoder_out[i].T.astype(np.float32)),
            "predT": np.ascontiguousarray(pT[:, eo]),
            "W": Wc,
            "bias": bias,
            "ones": ones,
            "sel": sel,
        })
    return maps


def run(encoder_out, predictor_out, W, b, trace=False, tmpdir=None):
    from concourse.bass_utils import run_bass_kernel_spmd

    nc = _get_compiled()
    maps = _in_maps(encoder_out, predictor_out, W, b)
    res = run_bass_kernel_spmd(
        nc, maps, list(range(B)), trace=trace,
        **({"tmpdir": tmpdir} if tmpdir else {}))
    outs = np.stack([res.results[i]["out"].reshape(T, U1, V) for i in range(B)])
    return outs, res


def kernel(encoder_out, predictor_out, W, b):
    outs, _ = run(encoder_out, predictor_out, W, b)
    return outs
